# revision 34
# baseline (speedup 1.0000x reference)
"""Trainium2 Bass kernel for sparse-depth k-NN (nn_Dist).

For every pixel q of a 96x128 grid, find the 4 nearest valid pixels
(S > 0.001) by Euclidean distance, with jax.lax.top_k tie-breaking
(equal distance -> lowest linear index first).

Device algorithm (8 NeuronCores, SPMD over query rows, 1536 queries/core,
12 tiles of 128 queries = one pixel row per tile): the TensorEngine
computes, for each query q and candidate c,

    key(q, c) = 640 * (2*qx*cx + 2*qy*cy - cx^2 - cy^2) - idx_c
              = 640 * (-|q-c|^2 + qx^2 + qy^2) - idx_c

as a bf16 matmul (K=7: every factor split into bf16-exact integer parts)
accumulated in fp32 PSUM. Every product / partial sum stays an exact fp32
integer under either PE accumulation direction, so keys are EXACT. Keys
order candidates per query by (distance asc, index asc) — exactly
jax.lax.top_k order — and are unique, so the VectorEngine MAX8 instruction
alone (top-8 values per partition, read straight from PSUM) yields the
top-4; the host decodes idx = (-key) mod 640.

Candidate pruning: each tile is one pixel row y0. The exact 4th-NN
distance at every 2x2-cell center plus the cell radius upper-bounds every
query's 4th-NN distance (triangle inequality), giving a per-row radius
R(y0); any candidate with |cy - y0| > R cannot be in that row's top-4, so
each tile only scores its y-band of candidates (~4x fewer MAX8 columns).

For candidate counts > 640 the scaled key would overflow the 2^24
exact-integer range, so a fallback variant computes the unscaled score
(K=4 bf16 matmul) and uses MAX8 + MAX_INDEX (HW tie-break = first
occurrence = lowest index, verified exact vs top_k on HW).

Raw Bass with explicit semaphores: the Tile scheduler emits multiple
embedded sync-waits on Matmult instructions, which walrus codegen rejects
(the PE LDWEIGHTS struct holds one); standalone wait_ge ops avoid that.
"""

import numpy as np

H, W = 96, 128
N = H * W                    # 12288 queries
N_NEIGHBORS = 4
V_THRESH = 0.001
N_CORES = 8
QPC = N // N_CORES           # 1536 queries per core
P = 128                      # partitions
TILES = QPC // P             # 12 query tiles (pixel rows) per core
KEY_M = 640.0                # key multiplier; KEY_M*(25154+1) < 2^24
MAX_C_KEYED = 640            # idx < KEY_M and exactness both need C <= 640
MAX_C = 4096                 # PSUM free-dim capacity (fp32)
PSUM_WORDS = 4096            # fp32 words per partition in all 8 banks

_module_cache = {}
LAST_RESULTS = None  # BassKernelResults of the most recent device run


def _build_module(C, keyed, per_tile_b):
    """Raw-Bass module for C candidate columns.

    keyed=True : K=7 matmul of index-encoded keys, MAX8 only, fp32 out.
    keyed=False: K=4 matmul of plain scores, MAX8 + MAX_INDEX, uint32 out.
    per_tile_b : each tile has its own C candidate columns (y-band pruning).
    """
    import concourse.bass as bass
    import concourse.mybir as mybir

    f32 = mybir.dt.float32
    u32 = mybir.dt.uint32
    bf16 = mybir.dt.bfloat16
    K = 7 if keyed else 4
    nb = TILES if per_tile_b else 1

    slot_words = 512
    while slot_words < C:
        slot_words *= 2
    n_slots = max(1, PSUM_WORDS // slot_words)

    # Skip the const-AP memsets + trailing all-engine barrier that
    # Bass.__init__ emits: this kernel never reads the const tiles, and
    # the Block-entry handshake already orders user code after the engine
    # preambles. Saves ~0.5us of NEFF preamble.
    _ob = bass.Bass.all_engine_barrier
    _om = bass.BassEitherVectorEngine.memset
    bass.Bass.all_engine_barrier = lambda self, **kw: None
    bass.BassEitherVectorEngine.memset = lambda self, ap, c: None
    try:
        nc = bass.Bass(enable_partition_id=False, enable_asserts=False,
                       monotonic_sem_count=0)
    finally:
        bass.Bass.all_engine_barrier = _ob
        bass.BassEitherVectorEngine.memset = _om
    AB = nc.dram_tensor("AB", [K, QPC + nb * C], bf16, kind="ExternalInput")
    out_dt = f32 if keyed else u32
    OUT = nc.dram_tensor("OUT", [P, TILES * 8], out_dt, kind="ExternalOutput")

    with (
        nc.sbuf_tensor("ab_t", [K, QPC + nb * C], bf16) as ab_t,
        nc.sbuf_tensor("mx_all", [P, TILES * 8], f32) as mx_all,
        nc.psum_tensor("ps", [P, PSUM_WORDS], f32) as ps,
        nc.semaphore("dma_in") as dma_in,
        nc.semaphore("dma_in2") as dma_in2,
        nc.semaphore("dma_in3") as dma_in3,
        nc.semaphore("pe_sem") as pe_sem,
        nc.semaphore("dve_sem") as dve_sem,
        nc.semaphore("dma_out") as dma_out,
    ):
        if keyed:
            _emit(nc, C, nb, slot_words, n_slots, AB, OUT, ab_t, mx_all,
                  None, ps, dma_in, dma_in2, dma_in3, pe_sem, dve_sem,
                  dma_out, keyed=True)
        else:
            with nc.sbuf_tensor("ix_all", [P, TILES * 8], u32) as ix_all:
                _emit(nc, C, nb, slot_words, n_slots, AB, OUT, ab_t, mx_all,
                      ix_all, ps, dma_in, dma_in2, dma_in3, pe_sem, dve_sem,
                      dma_out, keyed=False)
    return nc


def _emit(nc, C, nb, slot_words, n_slots, AB, OUT, ab_t, mx_all, ix_all,
          ps, dma_in, dma_in2, dma_in3, pe_sem, dve_sem, dma_out, keyed):
    dve_per_tile = 1 if keyed else 2
    out_sb = mx_all if keyed else ix_all
    # column layout: nb>1 -> [B0 | A0-1 | B1-5 | A2-11 | B6-11]; the first
    # (tiny) DMA covers tiles 0-1's A plus B0 so the pipeline starts as
    # early as possible, a parallel ACT-queue DMA brings B1-5, and a
    # second SP-queue DMA the rest. nb==1 -> [A | B], one DMA.
    ah = 2 * P                                    # A columns in dma1
    cut = C + ah if nb > 1 else QPC + C           # end of [B0 | A0-1]
    cut2 = cut + (TILES // 2 - 1) * C             # end of [.. B1-5]
    arest = (TILES - 2) * P                       # A2-11

    def aoff(i):
        if nb == 1:
            return i * P
        return C + i * P if i < 2 else cut2 + (i - 2) * P

    def boff(i):
        if nb == 1:
            return QPC
        if i == 0:
            return 0
        if i < TILES // 2:
            return cut + (i - 1) * C
        return cut2 + arest + (i - TILES // 2) * C

    n_out = 4                                     # output DMA chunks
    step = TILES // n_out

    with nc.Block() as block:

        if nb > 1:

            @block.scalar
            def _(scalar):
                # parallel HWDGE queue for the next candidate blocks
                scalar.dma_start(ab_t[:, cut:cut2], AB[:, cut:cut2]) \
                    .then_inc(dma_in2, 16)

        @block.sync
        def _(sync):
            sync.dma_start(ab_t[:, :cut], AB[:, :cut]).then_inc(dma_in, 16)
            if nb > 1:
                sync.dma_start(ab_t[:, cut2:], AB[:, cut2:]).then_inc(
                    dma_in3, 16)
            # ship results in chunks so the final DMA is small; no final
            # wait on dma_out - the module epilogue drains the DMA queues,
            # overlapping the last chunk's completion latency with the
            # semaphore-reset storm.
            for k in range(n_out):
                sync.wait_ge(dve_sem, dve_per_tile * step * (k + 1))
                sl = slice(step * 8 * k, step * 8 * (k + 1))
                sync.dma_start(OUT[:, sl], out_sb[:, sl]).then_inc(
                    dma_out, 16)

        @block.tensor
        def _(tensor):
            # probe: timestamps when the PE body becomes executable
            tensor.wait_ge(dve_sem, 0)
            tensor.wait_ge(dma_in, 16)
            for i in range(TILES):
                if i == 1 and nb > 1:
                    tensor.wait_ge(dma_in2, 16)
                if i == 2 and nb > 1:
                    tensor.wait_ge(dma_in3, 16)
                if i >= n_slots:
                    # PSUM slot (i % n_slots) must be drained by the DVE
                    tensor.wait_ge(
                        dve_sem, dve_per_tile * (i - n_slots + 1))
                base = (i % n_slots) * slot_words
                lhsT = ab_t[:, aoff(i):aoff(i) + P]
                last = None
                for j0 in range(0, C, 512):
                    j1 = min(j0 + 512, C)
                    last = tensor.matmul(
                        ps[:, base + j0:base + j1],
                        lhsT,
                        ab_t[:, boff(i) + j0:boff(i) + j1],
                    )
                last.then_inc(pe_sem)

        @block.vector
        def _(vector):
            for i in range(TILES):
                vector.wait_ge(pe_sem, i + 1)
                base = (i % n_slots) * slot_words
                sc_i = ps[:, base:base + C]
                vector.max(
                    out=mx_all[:, i * 8:(i + 1) * 8], in_=sc_i
                ).then_inc(dve_sem)
                if not keyed:
                    vector.wait_ge(dve_sem, 2 * i + 1)
                    vector.max_index(
                        out=ix_all[:, i * 8:(i + 1) * 8],
                        in_max=mx_all[:, i * 8:(i + 1) * 8],
                        in_values=sc_i,
                    ).then_inc(dve_sem)


def _get_module(C, keyed, per_tile_b):
    key = (C, keyed, per_tile_b)
    if key not in _module_cache:
        _module_cache[key] = _build_module(C, keyed, per_tile_b)
    return _module_cache[key]


def _run_device(ABmat, C, keyed, per_tile_b):
    """ABmat: list of [K, cols] bf16 per core -> (N, 8) out values."""
    from concourse.bass_utils import run_bass_kernel_spmd

    nc = _get_module(C, keyed, per_tile_b)
    in_maps = [{"AB": ab} for ab in ABmat]
    res = run_bass_kernel_spmd(nc, in_maps, core_ids=list(range(N_CORES)))
    global LAST_RESULTS
    LAST_RESULTS = res
    outs = []
    for r in res.results:
        o = r["OUT"].reshape(P, TILES, 8)          # [p, tile, rank]
        outs.append(o.transpose(1, 0, 2).reshape(QPC, 8))
    return np.concatenate(outs, axis=0)


def _query_features(keyed):
    """Per-query lhsT rows [K, N] as float (bf16-exact integer values)."""
    q = np.arange(N)
    qx = (q % W).astype(np.float64)
    qy = (q // W).astype(np.float64)
    if keyed:
        # 2*KEY_M*qx = 1280*qx split as 20480*(qx>>4) + 1280*(qx&15).
        # K order chosen so partial sums stay exact fp32 integers under
        # either PE accumulation direction.
        rows = [
            20480.0 * np.floor(qx / 16),   # * cx
            np.full(N, -65536.0),          # * v2
            20480.0 * np.floor(qy / 16),   # * cy
            1280.0 * (qx % 16),            # * cx
            1280.0 * (qy % 16),            # * cy
            np.full(N, -256.0),            # * v1
            np.full(N, -1.0),              # * v0
        ]
    else:
        rows = [
            2.0 * qx,                      # * cx
            2.0 * qy,                      # * cy
            np.full(N, -256.0),            # * w1
            np.full(N, -1.0),              # * w0
        ]
    return np.stack(rows)                  # [K, N]


def _cand_features(keyed, C, cand_idx):
    """Per-candidate rhs rows [K, C] incl. padding columns.

    cand_idx: pixel indices of this block's candidates (ascending)."""
    n = cand_idx.size
    cx = (cand_idx % W).astype(np.float64)
    cy = (cand_idx // W).astype(np.float64)
    Bm = np.zeros((7 if keyed else 4, C), np.float64)
    if keyed:
        v = KEY_M * (cx * cx + cy * cy) + np.arange(n, dtype=np.float64)
        assert n <= KEY_M and v.max(initial=0) < 2 ** 24
        Bm[0, :n] = cx
        Bm[1, :n] = np.floor(v / 65536)
        Bm[2, :n] = cy
        Bm[3, :n] = cx
        Bm[4, :n] = cy
        Bm[5, :n] = np.floor(v / 256) % 256
        Bm[6, :n] = v % 256
        # padding: key = -(65536+256+1)*255 = -16777215 < any real key
        Bm[1, n:] = 255.0
        Bm[5, n:] = 255.0
        Bm[6, n:] = 255.0
    else:
        w = cx * cx + cy * cy              # <= 25154
        Bm[0, :n] = cx
        Bm[1, :n] = cy
        Bm[2, :n] = np.floor(w / 256)
        Bm[3, :n] = w % 256
        # padding: score = -65535 < real score min (-25154)
        Bm[2, n:] = 255.0
        Bm[3, n:] = 255.0
    return Bm


def _row_radius(valid_idx):
    """Per pixel row y0: radius R such that every query in row y0 has its
    4 nearest valid pixels within |cy - y0| <= R. Bound: exact 4th-NN
    distance at every 2x2-cell center plus the cell radius (triangle
    inequality with the center's four nearest as witnesses)."""
    cx = (valid_idx % W).astype(np.float64)
    cy = (valid_idx // W).astype(np.float64)
    ccx = np.arange(W // 2) * 2 + 0.5
    ccy = np.arange(H // 2) * 2 + 0.5
    dx = ccx[None, :, None] - cx[None, None, :]
    dy = ccy[:, None, None] - cy[None, None, :]
    d = np.sqrt(dx * dx + dy * dy)                # [H/2, W/2, n_valid]
    d4 = np.partition(d, N_NEIGHBORS - 1, axis=2)[:, :, N_NEIGHBORS - 1]
    bound = d4.max(axis=1) + np.sqrt(0.5)         # per cell row
    return np.ceil(bound[np.arange(H) // 2]).astype(np.int64)


def _host_fallback(flat, valid_idx):
    """Exact numpy replication of the reference for degenerate inputs."""
    q = np.arange(N)
    qx = (q % W).astype(np.float32)
    qy = (q // W).astype(np.float32)
    cx = (valid_idx % W).astype(np.float32)
    cy = (valid_idx // W).astype(np.float32)
    pos4 = np.empty((N, N_NEIGHBORS), np.int64)
    chunk = 512
    for s in range(0, N, chunk):
        e = min(s + chunk, N)
        dx = qx[s:e, None] - cx[None, :]
        dy = qy[s:e, None] - cy[None, :]
        sc = np.full((e - s, N), -np.inf, np.float32)
        sc[:, valid_idx] = -(dx * dx + dy * dy)
        order = np.argsort(-sc, axis=1, kind="stable")
        pos4[s:e] = order[:, :N_NEIGHBORS]
    return pos4  # already pixel indices (full-N score rows)


def _pack_ab(Arows, Bblocks, C):
    """Assemble per-core AB matrices. Bblocks: [N_CORES][nb] of [K, C].
    nb>1 layout: [B0 | A | B1..B11]; nb==1 layout: [A | B]."""
    import ml_dtypes

    Kdim = Arows.shape[0]
    ht = TILES // 2
    ah = 2 * P
    arest = (TILES - 2) * P
    ABmat = []
    for c in range(N_CORES):
        nb = len(Bblocks[c])
        ab = np.empty((Kdim, QPC + nb * C), np.float64)
        A = Arows[:, c * QPC:(c + 1) * QPC]
        if nb == 1:
            ab[:, :QPC] = A
            ab[:, QPC:] = Bblocks[c][0]
        else:
            # [B0 | A0-1 | B1-5 | A2-11 | B6-11] (see _emit)
            cut = C + ah
            cut2 = cut + (ht - 1) * C
            ab[:, :C] = Bblocks[c][0]
            ab[:, C:cut] = A[:, :ah]
            for i in range(1, ht):
                ab[:, cut + (i - 1) * C:cut + i * C] = Bblocks[c][i]
            ab[:, cut2:cut2 + arest] = A[:, ah:]
            for i in range(ht, nb):
                o = cut2 + arest + (i - ht) * C
                ab[:, o:o + C] = Bblocks[c][i]
        ABmat.append(np.ascontiguousarray(ab.astype(ml_dtypes.bfloat16)))
    return ABmat


def kernel(S):
    S = np.asarray(S)
    flat = S.reshape(-1).astype(np.float32)
    valid_idx = np.flatnonzero(flat > V_THRESH)
    n_valid = int(valid_idx.size)

    if n_valid < 8 or n_valid > MAX_C:
        args_nq = _host_fallback(flat, valid_idx)
    else:
        cy = valid_idx // W
        R = _row_radius(valid_idx)                         # (H,)
        bands = [valid_idx[(cy >= y0 - R[y0]) & (cy <= y0 + R[y0])]
                 for y0 in range(H)]
        c_band = max(b.size for b in bands)
        C_tile = max(64, ((c_band + 15) // 16) * 16)

        if C_tile <= MAX_C_KEYED and C_tile < n_valid:
            # banded keyed path: per-tile candidate y-bands
            Arows = _query_features(True)
            Bblocks = [[_cand_features(True, C_tile, bands[12 * c + i])
                        for i in range(TILES)] for c in range(N_CORES)]
            out = _run_device(_pack_ab(Arows, Bblocks, C_tile),
                              C_tile, True, True)
            keys = out[:, :N_NEIGHBORS].astype(np.float64)
            pos = np.mod(-keys, KEY_M).astype(np.int64)    # (N, 4) local
            band_arr = np.zeros((H, C_tile), np.int64)
            for y0 in range(H):
                band_arr[y0, :bands[y0].size] = bands[y0]
            args_nq = band_arr[(np.arange(N) // W)[:, None], pos]
        else:
            # shared full candidate set
            C = max(P, ((n_valid + P - 1) // P) * P)
            keyed = C <= MAX_C_KEYED
            Arows = _query_features(keyed)
            Bblock = _cand_features(keyed, C, valid_idx)
            out = _run_device(_pack_ab(Arows, [[Bblock]] * N_CORES, C),
                              C, keyed, False)
            if keyed:
                keys = out[:, :N_NEIGHBORS].astype(np.float64)
                pos = np.mod(-keys, KEY_M).astype(np.int64)
            else:
                pos = out[:, :N_NEIGHBORS].astype(np.int64)
            args_nq = valid_idx[pos]

    args = args_nq.T.astype(np.int32)[None]                # (1, 4, N)
    ipc = np.empty((1, 2, N_NEIGHBORS, N), np.float32)
    ipc[0, 0] = (args[0] % W).astype(np.float32)
    ipc[0, 1] = (args[0] // W).astype(np.float32)
    return ipc, args


# revision 35
# speedup vs baseline: 1.0669x; 1.0669x over previous
"""Trainium2 Bass kernel for sparse-depth k-NN (nn_Dist).

For every pixel q of a 96x128 grid, find the 4 nearest valid pixels
(S > 0.001) by Euclidean distance, with jax.lax.top_k tie-breaking
(equal distance -> lowest linear index first).

Device algorithm (8 NeuronCores, SPMD over query rows, 1536 queries/core,
12 tiles of 128 queries = one pixel row per tile): the TensorEngine
computes, for each query q and candidate c,

    key(q, c) = 640 * (2*qx*cx + 2*qy*cy - cx^2 - cy^2) - idx_c
              = 640 * (-|q-c|^2 + qx^2 + qy^2) - idx_c

as a bf16 matmul (K=7: every factor split into bf16-exact integer parts)
accumulated in fp32 PSUM. Every product / partial sum stays an exact fp32
integer under either PE accumulation direction, so keys are EXACT. Keys
order candidates per query by (distance asc, index asc) — exactly
jax.lax.top_k order — and are unique, so the VectorEngine MAX8 instruction
alone (top-8 values per partition, read straight from PSUM) yields the
top-4; the host decodes idx = (-key) mod 640.

Candidate pruning: each tile is one pixel row y0. The exact 4th-NN
distance at every 2x2-cell center plus the cell radius upper-bounds every
query's 4th-NN distance (triangle inequality), giving a per-row radius
R(y0); any candidate with |cy - y0| > R cannot be in that row's top-4, so
each tile only scores its y-band of candidates (~4x fewer MAX8 columns).

For candidate counts > 640 the scaled key would overflow the 2^24
exact-integer range, so a fallback variant computes the unscaled score
(K=4 bf16 matmul) and uses MAX8 + MAX_INDEX (HW tie-break = first
occurrence = lowest index, verified exact vs top_k on HW).

Raw Bass with explicit semaphores: the Tile scheduler emits multiple
embedded sync-waits on Matmult instructions, which walrus codegen rejects
(the PE LDWEIGHTS struct holds one); standalone wait_ge ops avoid that.
"""

import numpy as np

H, W = 96, 128
N = H * W                    # 12288 queries
N_NEIGHBORS = 4
V_THRESH = 0.001
N_CORES = 8
QPC = N // N_CORES           # 1536 queries per core
P = 128                      # partitions
TILES = QPC // P             # 12 query tiles (pixel rows) per core
KEY_M = 640.0                # key multiplier; KEY_M*(25154+1) < 2^24
MAX_C_KEYED = 640            # idx < KEY_M and exactness both need C <= 640
MAX_C = 4096                 # PSUM free-dim capacity (fp32)
PSUM_WORDS = 4096            # fp32 words per partition in all 8 banks

_module_cache = {}
LAST_RESULTS = None  # BassKernelResults of the most recent device run


def _build_module(C, keyed, per_tile_b):
    """Raw-Bass module for C candidate columns.

    keyed=True : K=7 matmul of index-encoded keys, MAX8 only, fp32 out.
    keyed=False: K=4 matmul of plain scores, MAX8 + MAX_INDEX, uint32 out.
    per_tile_b : each tile has its own C candidate columns (y-band pruning).
    """
    import concourse.bass as bass
    import concourse.mybir as mybir

    f32 = mybir.dt.float32
    u32 = mybir.dt.uint32
    bf16 = mybir.dt.bfloat16
    K = 7 if keyed else 4
    nb = TILES if per_tile_b else 1

    slot_words = 512
    while slot_words < C:
        slot_words *= 2
    n_slots = max(1, PSUM_WORDS // slot_words)

    # Skip the const-AP memsets + trailing all-engine barrier that
    # Bass.__init__ emits: this kernel never reads the const tiles, and
    # the Block-entry handshake already orders user code after the engine
    # preambles. Saves ~0.5us of NEFF preamble.
    _ob = bass.Bass.all_engine_barrier
    _om = bass.BassEitherVectorEngine.memset
    bass.Bass.all_engine_barrier = lambda self, **kw: None
    bass.BassEitherVectorEngine.memset = lambda self, ap, c: None
    try:
        nc = bass.Bass(enable_partition_id=False, enable_asserts=False,
                       monotonic_sem_count=0)
    finally:
        bass.Bass.all_engine_barrier = _ob
        bass.BassEitherVectorEngine.memset = _om
    AB = nc.dram_tensor("AB", [K, QPC + nb * C], bf16, kind="ExternalInput")
    out_dt = f32 if keyed else u32
    OUT = nc.dram_tensor("OUT", [P, TILES * 8], out_dt, kind="ExternalOutput")

    with (
        nc.sbuf_tensor("ab_t", [K, QPC + nb * C], bf16) as ab_t,
        nc.sbuf_tensor("mx_all", [P, TILES * 8], f32) as mx_all,
        nc.psum_tensor("ps", [P, PSUM_WORDS], f32) as ps,
        nc.semaphore("dma_in") as dma_in,
        nc.semaphore("dma_in2") as dma_in2,
        nc.semaphore("dma_in3") as dma_in3,
        nc.semaphore("pe_sem") as pe_sem,
        nc.semaphore("dve_sem") as dve_sem,
        nc.semaphore("dma_out") as dma_out,
    ):
        if keyed:
            _emit(nc, C, nb, slot_words, n_slots, AB, OUT, ab_t, mx_all,
                  None, ps, dma_in, dma_in2, dma_in3, pe_sem, dve_sem,
                  dma_out, keyed=True)
        else:
            with nc.sbuf_tensor("ix_all", [P, TILES * 8], u32) as ix_all:
                _emit(nc, C, nb, slot_words, n_slots, AB, OUT, ab_t, mx_all,
                      ix_all, ps, dma_in, dma_in2, dma_in3, pe_sem, dve_sem,
                      dma_out, keyed=False)
    return nc


def _emit(nc, C, nb, slot_words, n_slots, AB, OUT, ab_t, mx_all, ix_all,
          ps, dma_in, dma_in2, dma_in3, pe_sem, dve_sem, dma_out, keyed):
    dve_per_tile = 1 if keyed else 2
    out_sb = mx_all if keyed else ix_all
    # column layout: nb>1 -> [B0 | A0-5 | B1-5 | A6-11 | B6-11]; the first
    # (small) DMA covers everything tiles 0..5 need from A plus B0, a
    # parallel ACT-queue DMA brings B1-5, and a second SP-queue DMA the
    # rest. nb==1 -> [A | B], one DMA.
    ah = TILES // 2 * P                           # A columns in dma1
    cut = C + ah if nb > 1 else QPC + C           # end of [B0 | A0-1]
    cut2 = cut + (TILES // 2 - 1) * C             # end of [.. B1-5]
    arest = TILES // 2 * P                        # A6-11

    def aoff(i):
        if nb == 1:
            return i * P
        return C + i * P if i < TILES // 2 else cut2 + (i - TILES // 2) * P

    def boff(i):
        if nb == 1:
            return QPC
        if i == 0:
            return 0
        if i < TILES // 2:
            return cut + (i - 1) * C
        return cut2 + arest + (i - TILES // 2) * C

    n_out = 4                                     # output DMA chunks
    step = TILES // n_out

    with nc.Block() as block:

        if nb > 1:

            @block.scalar
            def _(scalar):
                # parallel HWDGE queue for the next candidate blocks
                scalar.dma_start(ab_t[:, cut:cut2], AB[:, cut:cut2]) \
                    .then_inc(dma_in2, 16)

        @block.sync
        def _(sync):
            sync.dma_start(ab_t[:, :cut], AB[:, :cut]).then_inc(dma_in, 16)
            if nb > 1:
                sync.dma_start(ab_t[:, cut2:], AB[:, cut2:]).then_inc(
                    dma_in3, 16)
            # ship results in chunks so the final DMA is small; no final
            # wait on dma_out - the module epilogue drains the DMA queues,
            # overlapping the last chunk's completion latency with the
            # semaphore-reset storm.
            for k in range(n_out):
                sync.wait_ge(dve_sem, dve_per_tile * step * (k + 1))
                sl = slice(step * 8 * k, step * 8 * (k + 1))
                sync.dma_start(OUT[:, sl], out_sb[:, sl]).then_inc(
                    dma_out, 16)

        @block.tensor
        def _(tensor):
            # probe: timestamps when the PE body becomes executable
            tensor.wait_ge(dve_sem, 0)
            tensor.wait_ge(dma_in, 16)
            for i in range(TILES):
                if i == 1 and nb > 1:
                    tensor.wait_ge(dma_in2, 16)
                if i == TILES // 2 and nb > 1:
                    tensor.wait_ge(dma_in3, 16)
                if i >= n_slots:
                    # PSUM slot (i % n_slots) must be drained by the DVE
                    tensor.wait_ge(
                        dve_sem, dve_per_tile * (i - n_slots + 1))
                base = (i % n_slots) * slot_words
                lhsT = ab_t[:, aoff(i):aoff(i) + P]
                last = None
                for j0 in range(0, C, 512):
                    j1 = min(j0 + 512, C)
                    last = tensor.matmul(
                        ps[:, base + j0:base + j1],
                        lhsT,
                        ab_t[:, boff(i) + j0:boff(i) + j1],
                    )
                last.then_inc(pe_sem)

        @block.vector
        def _(vector):
            for i in range(TILES):
                vector.wait_ge(pe_sem, i + 1)
                base = (i % n_slots) * slot_words
                sc_i = ps[:, base:base + C]
                vector.max(
                    out=mx_all[:, i * 8:(i + 1) * 8], in_=sc_i
                ).then_inc(dve_sem)
                if not keyed:
                    vector.wait_ge(dve_sem, 2 * i + 1)
                    vector.max_index(
                        out=ix_all[:, i * 8:(i + 1) * 8],
                        in_max=mx_all[:, i * 8:(i + 1) * 8],
                        in_values=sc_i,
                    ).then_inc(dve_sem)


def _get_module(C, keyed, per_tile_b):
    key = (C, keyed, per_tile_b)
    if key not in _module_cache:
        _module_cache[key] = _build_module(C, keyed, per_tile_b)
    return _module_cache[key]


def _run_device(ABmat, C, keyed, per_tile_b):
    """ABmat: list of [K, cols] bf16 per core -> (N, 8) out values."""
    from concourse.bass_utils import run_bass_kernel_spmd

    nc = _get_module(C, keyed, per_tile_b)
    in_maps = [{"AB": ab} for ab in ABmat]
    res = run_bass_kernel_spmd(nc, in_maps, core_ids=list(range(N_CORES)))
    global LAST_RESULTS
    LAST_RESULTS = res
    outs = []
    for r in res.results:
        o = r["OUT"].reshape(P, TILES, 8)          # [p, tile, rank]
        outs.append(o.transpose(1, 0, 2).reshape(QPC, 8))
    return np.concatenate(outs, axis=0)


def _query_features(keyed):
    """Per-query lhsT rows [K, N] as float (bf16-exact integer values)."""
    q = np.arange(N)
    qx = (q % W).astype(np.float64)
    qy = (q // W).astype(np.float64)
    if keyed:
        # 2*KEY_M*qx = 1280*qx split as 20480*(qx>>4) + 1280*(qx&15).
        # K order chosen so partial sums stay exact fp32 integers under
        # either PE accumulation direction.
        rows = [
            20480.0 * np.floor(qx / 16),   # * cx
            np.full(N, -65536.0),          # * v2
            20480.0 * np.floor(qy / 16),   # * cy
            1280.0 * (qx % 16),            # * cx
            1280.0 * (qy % 16),            # * cy
            np.full(N, -256.0),            # * v1
            np.full(N, -1.0),              # * v0
        ]
    else:
        rows = [
            2.0 * qx,                      # * cx
            2.0 * qy,                      # * cy
            np.full(N, -256.0),            # * w1
            np.full(N, -1.0),              # * w0
        ]
    return np.stack(rows)                  # [K, N]


def _cand_features(keyed, C, cand_idx):
    """Per-candidate rhs rows [K, C] incl. padding columns.

    cand_idx: pixel indices of this block's candidates (ascending)."""
    n = cand_idx.size
    cx = (cand_idx % W).astype(np.float64)
    cy = (cand_idx // W).astype(np.float64)
    Bm = np.zeros((7 if keyed else 4, C), np.float64)
    if keyed:
        v = KEY_M * (cx * cx + cy * cy) + np.arange(n, dtype=np.float64)
        assert n <= KEY_M and v.max(initial=0) < 2 ** 24
        Bm[0, :n] = cx
        Bm[1, :n] = np.floor(v / 65536)
        Bm[2, :n] = cy
        Bm[3, :n] = cx
        Bm[4, :n] = cy
        Bm[5, :n] = np.floor(v / 256) % 256
        Bm[6, :n] = v % 256
        # padding: key = -(65536+256+1)*255 = -16777215 < any real key
        Bm[1, n:] = 255.0
        Bm[5, n:] = 255.0
        Bm[6, n:] = 255.0
    else:
        w = cx * cx + cy * cy              # <= 25154
        Bm[0, :n] = cx
        Bm[1, :n] = cy
        Bm[2, :n] = np.floor(w / 256)
        Bm[3, :n] = w % 256
        # padding: score = -65535 < real score min (-25154)
        Bm[2, n:] = 255.0
        Bm[3, n:] = 255.0
    return Bm


def _row_radius(valid_idx):
    """Per pixel row y0: radius R such that every query in row y0 has its
    4 nearest valid pixels within |cy - y0| <= R. Bound: exact 4th-NN
    distance at every 2x2-cell center plus the cell radius (triangle
    inequality with the center's four nearest as witnesses)."""
    cx = (valid_idx % W).astype(np.float64)
    cy = (valid_idx // W).astype(np.float64)
    ccx = np.arange(W // 2) * 2 + 0.5
    ccy = np.arange(H // 2) * 2 + 0.5
    dx = ccx[None, :, None] - cx[None, None, :]
    dy = ccy[:, None, None] - cy[None, None, :]
    d = np.sqrt(dx * dx + dy * dy)                # [H/2, W/2, n_valid]
    d4 = np.partition(d, N_NEIGHBORS - 1, axis=2)[:, :, N_NEIGHBORS - 1]
    bound = d4.max(axis=1) + np.sqrt(0.5)         # per cell row
    return np.ceil(bound[np.arange(H) // 2]).astype(np.int64)


def _host_fallback(flat, valid_idx):
    """Exact numpy replication of the reference for degenerate inputs."""
    q = np.arange(N)
    qx = (q % W).astype(np.float32)
    qy = (q // W).astype(np.float32)
    cx = (valid_idx % W).astype(np.float32)
    cy = (valid_idx // W).astype(np.float32)
    pos4 = np.empty((N, N_NEIGHBORS), np.int64)
    chunk = 512
    for s in range(0, N, chunk):
        e = min(s + chunk, N)
        dx = qx[s:e, None] - cx[None, :]
        dy = qy[s:e, None] - cy[None, :]
        sc = np.full((e - s, N), -np.inf, np.float32)
        sc[:, valid_idx] = -(dx * dx + dy * dy)
        order = np.argsort(-sc, axis=1, kind="stable")
        pos4[s:e] = order[:, :N_NEIGHBORS]
    return pos4  # already pixel indices (full-N score rows)


def _pack_ab(Arows, Bblocks, C):
    """Assemble per-core AB matrices. Bblocks: [N_CORES][nb] of [K, C].
    nb>1 layout: [B0 | A | B1..B11]; nb==1 layout: [A | B]."""
    import ml_dtypes

    Kdim = Arows.shape[0]
    ht = TILES // 2
    ah = ht * P
    arest = ht * P
    ABmat = []
    for c in range(N_CORES):
        nb = len(Bblocks[c])
        ab = np.empty((Kdim, QPC + nb * C), np.float64)
        A = Arows[:, c * QPC:(c + 1) * QPC]
        if nb == 1:
            ab[:, :QPC] = A
            ab[:, QPC:] = Bblocks[c][0]
        else:
            # [B0 | A0-5 | B1-5 | A6-11 | B6-11] (see _emit)
            cut = C + ah
            cut2 = cut + (ht - 1) * C
            ab[:, :C] = Bblocks[c][0]
            ab[:, C:cut] = A[:, :ah]
            for i in range(1, ht):
                ab[:, cut + (i - 1) * C:cut + i * C] = Bblocks[c][i]
            ab[:, cut2:cut2 + arest] = A[:, ah:]
            for i in range(ht, nb):
                o = cut2 + arest + (i - ht) * C
                ab[:, o:o + C] = Bblocks[c][i]
        ABmat.append(np.ascontiguousarray(ab.astype(ml_dtypes.bfloat16)))
    return ABmat


def kernel(S):
    S = np.asarray(S)
    flat = S.reshape(-1).astype(np.float32)
    valid_idx = np.flatnonzero(flat > V_THRESH)
    n_valid = int(valid_idx.size)

    if n_valid < 8 or n_valid > MAX_C:
        args_nq = _host_fallback(flat, valid_idx)
    else:
        cy = valid_idx // W
        R = _row_radius(valid_idx)                         # (H,)
        bands = [valid_idx[(cy >= y0 - R[y0]) & (cy <= y0 + R[y0])]
                 for y0 in range(H)]
        c_band = max(b.size for b in bands)
        C_tile = max(64, ((c_band + 15) // 16) * 16)

        if C_tile <= MAX_C_KEYED and C_tile < n_valid:
            # banded keyed path: per-tile candidate y-bands
            Arows = _query_features(True)
            Bblocks = [[_cand_features(True, C_tile, bands[12 * c + i])
                        for i in range(TILES)] for c in range(N_CORES)]
            out = _run_device(_pack_ab(Arows, Bblocks, C_tile),
                              C_tile, True, True)
            keys = out[:, :N_NEIGHBORS].astype(np.float64)
            pos = np.mod(-keys, KEY_M).astype(np.int64)    # (N, 4) local
            band_arr = np.zeros((H, C_tile), np.int64)
            for y0 in range(H):
                band_arr[y0, :bands[y0].size] = bands[y0]
            args_nq = band_arr[(np.arange(N) // W)[:, None], pos]
        else:
            # shared full candidate set
            C = max(P, ((n_valid + P - 1) // P) * P)
            keyed = C <= MAX_C_KEYED
            Arows = _query_features(keyed)
            Bblock = _cand_features(keyed, C, valid_idx)
            out = _run_device(_pack_ab(Arows, [[Bblock]] * N_CORES, C),
                              C, keyed, False)
            if keyed:
                keys = out[:, :N_NEIGHBORS].astype(np.float64)
                pos = np.mod(-keys, KEY_M).astype(np.int64)
            else:
                pos = out[:, :N_NEIGHBORS].astype(np.int64)
            args_nq = valid_idx[pos]

    args = args_nq.T.astype(np.int32)[None]                # (1, 4, N)
    ipc = np.empty((1, 2, N_NEIGHBORS, N), np.float32)
    ipc[0, 0] = (args[0] % W).astype(np.float32)
    ipc[0, 1] = (args[0] // W).astype(np.float32)
    return ipc, args


# revision 36
# speedup vs baseline: 1.0681x; 1.0011x over previous
"""Trainium2 Bass kernel for sparse-depth k-NN (nn_Dist).

For every pixel q of a 96x128 grid, find the 4 nearest valid pixels
(S > 0.001) by Euclidean distance, with jax.lax.top_k tie-breaking
(equal distance -> lowest linear index first).

Device algorithm (8 NeuronCores, SPMD over query rows, 1536 queries/core,
12 tiles of 128 queries = one pixel row per tile): the TensorEngine
computes, for each query q and candidate c,

    key(q, c) = 640 * (2*qx*cx + 2*qy*cy - cx^2 - cy^2) - idx_c
              = 640 * (-|q-c|^2 + qx^2 + qy^2) - idx_c

as a bf16 matmul (K=7: every factor split into bf16-exact integer parts)
accumulated in fp32 PSUM. Every product / partial sum stays an exact fp32
integer under either PE accumulation direction, so keys are EXACT. Keys
order candidates per query by (distance asc, index asc) — exactly
jax.lax.top_k order — and are unique, so the VectorEngine MAX8 instruction
alone (top-8 values per partition, read straight from PSUM) yields the
top-4; the host decodes idx = (-key) mod 640.

Candidate pruning: each tile is one pixel row y0. The exact 4th-NN
distance at every 2x2-cell center plus the cell radius upper-bounds every
query's 4th-NN distance (triangle inequality), giving a per-row radius
R(y0); any candidate with |cy - y0| > R cannot be in that row's top-4, so
each tile only scores its y-band of candidates (~4x fewer MAX8 columns).

For candidate counts > 640 the scaled key would overflow the 2^24
exact-integer range, so a fallback variant computes the unscaled score
(K=4 bf16 matmul) and uses MAX8 + MAX_INDEX (HW tie-break = first
occurrence = lowest index, verified exact vs top_k on HW).

Raw Bass with explicit semaphores: the Tile scheduler emits multiple
embedded sync-waits on Matmult instructions, which walrus codegen rejects
(the PE LDWEIGHTS struct holds one); standalone wait_ge ops avoid that.
"""

import numpy as np

H, W = 96, 128
N = H * W                    # 12288 queries
N_NEIGHBORS = 4
V_THRESH = 0.001
N_CORES = 8
QPC = N // N_CORES           # 1536 queries per core
P = 128                      # partitions
TILES = QPC // P             # 12 query tiles (pixel rows) per core
KEY_M = 640.0                # key multiplier; KEY_M*(25154+1) < 2^24
MAX_C_KEYED = 640            # idx < KEY_M and exactness both need C <= 640
MAX_C = 4096                 # PSUM free-dim capacity (fp32)
PSUM_WORDS = 4096            # fp32 words per partition in all 8 banks

_module_cache = {}
LAST_RESULTS = None  # BassKernelResults of the most recent device run


def _build_module(C, keyed, per_tile_b):
    """Raw-Bass module for C candidate columns.

    keyed=True : K=7 matmul of index-encoded keys, MAX8 only, fp32 out.
    keyed=False: K=4 matmul of plain scores, MAX8 + MAX_INDEX, uint32 out.
    per_tile_b : each tile has its own C candidate columns (y-band pruning).
    """
    import concourse.bass as bass
    import concourse.mybir as mybir

    f32 = mybir.dt.float32
    u32 = mybir.dt.uint32
    bf16 = mybir.dt.bfloat16
    K = 7 if keyed else 4
    nb = TILES if per_tile_b else 1

    slot_words = 512
    while slot_words < C:
        slot_words *= 2
    n_slots = max(1, PSUM_WORDS // slot_words)

    # Skip the const-AP memsets + trailing all-engine barrier that
    # Bass.__init__ emits: this kernel never reads the const tiles, and
    # the Block-entry handshake already orders user code after the engine
    # preambles. Saves ~0.5us of NEFF preamble.
    _ob = bass.Bass.all_engine_barrier
    _om = bass.BassEitherVectorEngine.memset
    bass.Bass.all_engine_barrier = lambda self, **kw: None
    bass.BassEitherVectorEngine.memset = lambda self, ap, c: None
    try:
        nc = bass.Bass(enable_partition_id=False, enable_asserts=False,
                       monotonic_sem_count=0)
    finally:
        bass.Bass.all_engine_barrier = _ob
        bass.BassEitherVectorEngine.memset = _om
    AB = nc.dram_tensor("AB", [K, QPC + nb * C], bf16, kind="ExternalInput")
    out_dt = f32 if keyed else u32
    OUT = nc.dram_tensor("OUT", [P, TILES * 8], out_dt, kind="ExternalOutput")

    with (
        nc.sbuf_tensor("ab_t", [K, QPC + nb * C], bf16) as ab_t,
        nc.sbuf_tensor("mx_all", [P, TILES * 8], f32) as mx_all,
        nc.psum_tensor("ps", [P, PSUM_WORDS], f32) as ps,
        nc.semaphore("dma_in") as dma_in,
        nc.semaphore("dma_in2") as dma_in2,
        nc.semaphore("dma_in3") as dma_in3,
        nc.semaphore("pe_sem") as pe_sem,
        nc.semaphore("dve_sem") as dve_sem,
        nc.semaphore("dma_out") as dma_out,
    ):
        if keyed:
            _emit(nc, C, nb, slot_words, n_slots, AB, OUT, ab_t, mx_all,
                  None, ps, dma_in, dma_in2, dma_in3, pe_sem, dve_sem,
                  dma_out, keyed=True)
        else:
            with nc.sbuf_tensor("ix_all", [P, TILES * 8], u32) as ix_all:
                _emit(nc, C, nb, slot_words, n_slots, AB, OUT, ab_t, mx_all,
                      ix_all, ps, dma_in, dma_in2, dma_in3, pe_sem, dve_sem,
                      dma_out, keyed=False)
    return nc


def _emit(nc, C, nb, slot_words, n_slots, AB, OUT, ab_t, mx_all, ix_all,
          ps, dma_in, dma_in2, dma_in3, pe_sem, dve_sem, dma_out, keyed):
    dve_per_tile = 1 if keyed else 2
    out_sb = mx_all if keyed else ix_all
    # column layout: nb>1 -> [B0 | A0-5 | B1-5 | A6-11 | B6-11]; the first
    # (small) DMA covers everything tiles 0..5 need from A plus B0, a
    # parallel ACT-queue DMA brings B1-5, and a second SP-queue DMA the
    # rest. nb==1 -> [A | B], one DMA.
    ah = TILES // 2 * P                           # A columns in dma1
    cut = C + ah if nb > 1 else QPC + C           # end of [B0 | A0-1]
    cut2 = cut + (TILES // 2 - 1) * C             # end of [.. B1-5]
    arest = TILES // 2 * P                        # A6-11

    def aoff(i):
        if nb == 1:
            return i * P
        return C + i * P if i < TILES // 2 else cut2 + (i - TILES // 2) * P

    def boff(i):
        if nb == 1:
            return QPC
        if i == 0:
            return 0
        if i < TILES // 2:
            return cut + (i - 1) * C
        return cut2 + arest + (i - TILES // 2) * C

    n_out = 4                                     # output DMA chunks
    step = TILES // n_out

    with nc.Block() as block:

        if nb > 1:

            @block.scalar
            def _(scalar):
                # parallel HWDGE queue for the next candidate blocks
                scalar.dma_start(ab_t[:, cut:cut2], AB[:, cut:cut2]) \
                    .then_inc(dma_in2, 16)

        @block.sync
        def _(sync):
            sync.dma_start(ab_t[:, :cut], AB[:, :cut]).then_inc(dma_in, 16)
            if nb > 1:
                sync.dma_start(ab_t[:, cut2:], AB[:, cut2:]).then_inc(
                    dma_in3, 16)
            # ship results in chunks so the final DMA is small; no final
            # wait on dma_out - the module epilogue drains the DMA queues,
            # overlapping the last chunk's completion latency with the
            # semaphore-reset storm.
            for k in range(n_out):
                sync.wait_ge(dve_sem, dve_per_tile * step * (k + 1))
                sl = slice(step * 8 * k, step * 8 * (k + 1))
                sync.dma_start(OUT[:, sl], out_sb[:, sl]).then_inc(
                    dma_out, 16)

        @block.tensor
        def _(tensor):
            tensor.wait_ge(dma_in, 16)
            for i in range(TILES):
                if i == 1 and nb > 1:
                    tensor.wait_ge(dma_in2, 16)
                if i == TILES // 2 and nb > 1:
                    tensor.wait_ge(dma_in3, 16)
                if i >= n_slots:
                    # PSUM slot (i % n_slots) must be drained by the DVE
                    tensor.wait_ge(
                        dve_sem, dve_per_tile * (i - n_slots + 1))
                base = (i % n_slots) * slot_words
                lhsT = ab_t[:, aoff(i):aoff(i) + P]
                last = None
                for j0 in range(0, C, 512):
                    j1 = min(j0 + 512, C)
                    last = tensor.matmul(
                        ps[:, base + j0:base + j1],
                        lhsT,
                        ab_t[:, boff(i) + j0:boff(i) + j1],
                    )
                last.then_inc(pe_sem)

        @block.vector
        def _(vector):
            for i in range(TILES):
                vector.wait_ge(pe_sem, i + 1)
                base = (i % n_slots) * slot_words
                sc_i = ps[:, base:base + C]
                vector.max(
                    out=mx_all[:, i * 8:(i + 1) * 8], in_=sc_i
                ).then_inc(dve_sem)
                if not keyed:
                    vector.wait_ge(dve_sem, 2 * i + 1)
                    vector.max_index(
                        out=ix_all[:, i * 8:(i + 1) * 8],
                        in_max=mx_all[:, i * 8:(i + 1) * 8],
                        in_values=sc_i,
                    ).then_inc(dve_sem)


def _get_module(C, keyed, per_tile_b):
    key = (C, keyed, per_tile_b)
    if key not in _module_cache:
        _module_cache[key] = _build_module(C, keyed, per_tile_b)
    return _module_cache[key]


def _run_device(ABmat, C, keyed, per_tile_b):
    """ABmat: list of [K, cols] bf16 per core -> (N, 8) out values."""
    from concourse.bass_utils import run_bass_kernel_spmd

    nc = _get_module(C, keyed, per_tile_b)
    in_maps = [{"AB": ab} for ab in ABmat]
    res = run_bass_kernel_spmd(nc, in_maps, core_ids=list(range(N_CORES)))
    global LAST_RESULTS
    LAST_RESULTS = res
    outs = []
    for r in res.results:
        o = r["OUT"].reshape(P, TILES, 8)          # [p, tile, rank]
        outs.append(o.transpose(1, 0, 2).reshape(QPC, 8))
    return np.concatenate(outs, axis=0)


def _query_features(keyed):
    """Per-query lhsT rows [K, N] as float (bf16-exact integer values)."""
    q = np.arange(N)
    qx = (q % W).astype(np.float64)
    qy = (q // W).astype(np.float64)
    if keyed:
        # 2*KEY_M*qx = 1280*qx split as 20480*(qx>>4) + 1280*(qx&15).
        # K order chosen so partial sums stay exact fp32 integers under
        # either PE accumulation direction.
        rows = [
            20480.0 * np.floor(qx / 16),   # * cx
            np.full(N, -65536.0),          # * v2
            20480.0 * np.floor(qy / 16),   # * cy
            1280.0 * (qx % 16),            # * cx
            1280.0 * (qy % 16),            # * cy
            np.full(N, -256.0),            # * v1
            np.full(N, -1.0),              # * v0
        ]
    else:
        rows = [
            2.0 * qx,                      # * cx
            2.0 * qy,                      # * cy
            np.full(N, -256.0),            # * w1
            np.full(N, -1.0),              # * w0
        ]
    return np.stack(rows)                  # [K, N]


def _cand_features(keyed, C, cand_idx):
    """Per-candidate rhs rows [K, C] incl. padding columns.

    cand_idx: pixel indices of this block's candidates (ascending)."""
    n = cand_idx.size
    cx = (cand_idx % W).astype(np.float64)
    cy = (cand_idx // W).astype(np.float64)
    Bm = np.zeros((7 if keyed else 4, C), np.float64)
    if keyed:
        v = KEY_M * (cx * cx + cy * cy) + np.arange(n, dtype=np.float64)
        assert n <= KEY_M and v.max(initial=0) < 2 ** 24
        Bm[0, :n] = cx
        Bm[1, :n] = np.floor(v / 65536)
        Bm[2, :n] = cy
        Bm[3, :n] = cx
        Bm[4, :n] = cy
        Bm[5, :n] = np.floor(v / 256) % 256
        Bm[6, :n] = v % 256
        # padding: key = -(65536+256+1)*255 = -16777215 < any real key
        Bm[1, n:] = 255.0
        Bm[5, n:] = 255.0
        Bm[6, n:] = 255.0
    else:
        w = cx * cx + cy * cy              # <= 25154
        Bm[0, :n] = cx
        Bm[1, :n] = cy
        Bm[2, :n] = np.floor(w / 256)
        Bm[3, :n] = w % 256
        # padding: score = -65535 < real score min (-25154)
        Bm[2, n:] = 255.0
        Bm[3, n:] = 255.0
    return Bm


def _row_radius(valid_idx):
    """Per pixel row y0: radius R such that every query in row y0 has its
    4 nearest valid pixels within |cy - y0| <= R. Bound: exact 4th-NN
    distance at every 2x2-cell center plus the cell radius (triangle
    inequality with the center's four nearest as witnesses)."""
    cx = (valid_idx % W).astype(np.float64)
    cy = (valid_idx // W).astype(np.float64)
    ccx = np.arange(W // 2) * 2 + 0.5
    ccy = np.arange(H // 2) * 2 + 0.5
    dx = ccx[None, :, None] - cx[None, None, :]
    dy = ccy[:, None, None] - cy[None, None, :]
    d = np.sqrt(dx * dx + dy * dy)                # [H/2, W/2, n_valid]
    d4 = np.partition(d, N_NEIGHBORS - 1, axis=2)[:, :, N_NEIGHBORS - 1]
    bound = d4.max(axis=1) + np.sqrt(0.5)         # per cell row
    return np.ceil(bound[np.arange(H) // 2]).astype(np.int64)


def _host_fallback(flat, valid_idx):
    """Exact numpy replication of the reference for degenerate inputs."""
    q = np.arange(N)
    qx = (q % W).astype(np.float32)
    qy = (q // W).astype(np.float32)
    cx = (valid_idx % W).astype(np.float32)
    cy = (valid_idx // W).astype(np.float32)
    pos4 = np.empty((N, N_NEIGHBORS), np.int64)
    chunk = 512
    for s in range(0, N, chunk):
        e = min(s + chunk, N)
        dx = qx[s:e, None] - cx[None, :]
        dy = qy[s:e, None] - cy[None, :]
        sc = np.full((e - s, N), -np.inf, np.float32)
        sc[:, valid_idx] = -(dx * dx + dy * dy)
        order = np.argsort(-sc, axis=1, kind="stable")
        pos4[s:e] = order[:, :N_NEIGHBORS]
    return pos4  # already pixel indices (full-N score rows)


def _pack_ab(Arows, Bblocks, C):
    """Assemble per-core AB matrices. Bblocks: [N_CORES][nb] of [K, C].
    nb>1 layout: [B0 | A | B1..B11]; nb==1 layout: [A | B]."""
    import ml_dtypes

    Kdim = Arows.shape[0]
    ht = TILES // 2
    ah = ht * P
    arest = ht * P
    ABmat = []
    for c in range(N_CORES):
        nb = len(Bblocks[c])
        ab = np.empty((Kdim, QPC + nb * C), np.float64)
        A = Arows[:, c * QPC:(c + 1) * QPC]
        if nb == 1:
            ab[:, :QPC] = A
            ab[:, QPC:] = Bblocks[c][0]
        else:
            # [B0 | A0-5 | B1-5 | A6-11 | B6-11] (see _emit)
            cut = C + ah
            cut2 = cut + (ht - 1) * C
            ab[:, :C] = Bblocks[c][0]
            ab[:, C:cut] = A[:, :ah]
            for i in range(1, ht):
                ab[:, cut + (i - 1) * C:cut + i * C] = Bblocks[c][i]
            ab[:, cut2:cut2 + arest] = A[:, ah:]
            for i in range(ht, nb):
                o = cut2 + arest + (i - ht) * C
                ab[:, o:o + C] = Bblocks[c][i]
        ABmat.append(np.ascontiguousarray(ab.astype(ml_dtypes.bfloat16)))
    return ABmat


def kernel(S):
    S = np.asarray(S)
    flat = S.reshape(-1).astype(np.float32)
    valid_idx = np.flatnonzero(flat > V_THRESH)
    n_valid = int(valid_idx.size)

    if n_valid < 8 or n_valid > MAX_C:
        args_nq = _host_fallback(flat, valid_idx)
    else:
        cy = valid_idx // W
        R = _row_radius(valid_idx)                         # (H,)
        bands = [valid_idx[(cy >= y0 - R[y0]) & (cy <= y0 + R[y0])]
                 for y0 in range(H)]
        c_band = max(b.size for b in bands)
        C_tile = max(64, ((c_band + 15) // 16) * 16)

        if C_tile <= MAX_C_KEYED and C_tile < n_valid:
            # banded keyed path: per-tile candidate y-bands
            Arows = _query_features(True)
            Bblocks = [[_cand_features(True, C_tile, bands[12 * c + i])
                        for i in range(TILES)] for c in range(N_CORES)]
            out = _run_device(_pack_ab(Arows, Bblocks, C_tile),
                              C_tile, True, True)
            keys = out[:, :N_NEIGHBORS].astype(np.float64)
            pos = np.mod(-keys, KEY_M).astype(np.int64)    # (N, 4) local
            band_arr = np.zeros((H, C_tile), np.int64)
            for y0 in range(H):
                band_arr[y0, :bands[y0].size] = bands[y0]
            args_nq = band_arr[(np.arange(N) // W)[:, None], pos]
        else:
            # shared full candidate set
            C = max(P, ((n_valid + P - 1) // P) * P)
            keyed = C <= MAX_C_KEYED
            Arows = _query_features(keyed)
            Bblock = _cand_features(keyed, C, valid_idx)
            out = _run_device(_pack_ab(Arows, [[Bblock]] * N_CORES, C),
                              C, keyed, False)
            if keyed:
                keys = out[:, :N_NEIGHBORS].astype(np.float64)
                pos = np.mod(-keys, KEY_M).astype(np.int64)
            else:
                pos = out[:, :N_NEIGHBORS].astype(np.int64)
            args_nq = valid_idx[pos]

    args = args_nq.T.astype(np.int32)[None]                # (1, 4, N)
    ipc = np.empty((1, 2, N_NEIGHBORS, N), np.float32)
    ipc[0, 0] = (args[0] % W).astype(np.float32)
    ipc[0, 1] = (args[0] // W).astype(np.float32)
    return ipc, args
